# revision 36
# baseline (speedup 1.0000x reference)
"""Self-contained Trainium2 kernel for nn_DecoderOnlyTransformer_10239202034008.

v2. Sharding: 8 cores = 4 pairs; pair p owns batch element p. Within a pair,
tokens are zigzag-chunk sharded (balanced causal work, SPMD-uniform program).
Residual stream feature-major (xT [D, SH]) fp32 + bf16 shadow; matmuls bf16.

Attention is split into a LOCAL pass (keys this core computed, straight from
SBUF) that overlaps the K/V AllGather, and a REMOTE pass over the peer's
half extracted from the gathered buffer with a partition-id-driven dynamic
DMA offset (program stays SPMD-uniform; rank asymmetry lives in host data:
peer_row index + remote masks).
"""

import math
import os
from contextlib import ExitStack
from types import SimpleNamespace

import numpy as np
import ml_dtypes

import concourse.bass as bass
import concourse.mybir as mybir
import concourse.tile as tile
from concourse import bacc
from concourse.bass_utils import run_bass_kernel_spmd

P = 128
F32 = mybir.dt.float32
BF16 = mybir.dt.bfloat16
U32 = mybir.dt.uint32


def make_cfg(B=4, S=2048, D=512, H=8, L=6, V=8000, FFM=4):
    c = SimpleNamespace()
    c.B, c.S, c.D, c.H, c.L, c.V, c.FFM = B, S, D, H, L, V, FFM
    c.HD = D // H
    c.FF = FFM * D
    c.NCH = S // P          # chunks per sequence (16)
    c.SH = S // 2           # tokens per core (1024)
    c.NSLOT = c.NCH // 2    # q/k slots per core (8)
    c.FB = D // P           # feature blocks (4)
    c.FFB = c.FF // P       # ff blocks (16)
    c.TB = c.SH // P        # local token blocks (8)
    c.NCORES = 2 * B
    c.VP = 8192             # padded vocab (64 * 128)
    c.NVB = c.VP // P       # vocab blocks (64)
    c.VCH = 16              # vocab DMA chunks (4 blocks each)
    c.eps = 1e-5
    c.inv_scale = 1.0 / math.sqrt(D)
    # zigzag chunk assignment, slots sorted by descending chunk id
    k = c.NCH // 2
    A = [x for x in range(0, k) if x % 2 == 0] + [x for x in range(k, c.NCH) if x % 2 == 1]
    Bs = [x for x in range(0, k) if x % 2 == 1] + [x for x in range(k, c.NCH) if x % 2 == 0]
    A = sorted(A, key=lambda x: -x)
    Bs = sorted(Bs, key=lambda x: -x)
    c.slot_chunks = {0: A, 1: Bs}
    # local step ks: k-chunk = own[ks]; active q-blocks 0..ks; block ks diag.
    for r in (0, 1):
        own = c.slot_chunks[r]
        for ks in range(c.NSLOT):
            assert sum(1 for x in own if x > own[ks]) == ks
    # remote step ks: k-chunk = peer[ks]; uniform M = ks+1 blocks; block ks is
    # full (rmask 1) or entirely non-causal (rmask 0) depending on rank.
    c.rmask = np.zeros((2, c.NSLOT), np.float32)
    for r in (0, 1):
        own, peer = c.slot_chunks[r], c.slot_chunks[1 - r]
        for ks in range(c.NSLOT):
            cnt = sum(1 for x in own if x > peer[ks])
            assert cnt in (ks, ks + 1)
            c.rmask[r, ks] = 1.0 if cnt == ks + 1 else 0.0
    return c


def _halves(n, w=512):
    out = []
    o = 0
    while o < n:
        out.append((o, min(w, n - o)))
        o += w
    return out


def build_program(c):
    nc = bacc.Bacc("TRN2", target_bir_lowering=False, debug=False,
                   num_devices=c.NCORES)

    x0T = nc.dram_tensor("x0T", [c.D, c.SH], F32, kind="ExternalInput").ap()
    tri = nc.dram_tensor("tri", [P, P], BF16, kind="ExternalInput").ap()
    ident = nc.dram_tensor("ident", [P, P], BF16, kind="ExternalInput").ap()
    rmask = nc.dram_tensor("rmask", [P, c.NSLOT], F32, kind="ExternalInput").ap()
    peer_row = nc.dram_tensor("peer_row", [1, 1], U32, kind="ExternalInput").ap()
    wq = nc.dram_tensor("wq", [c.L, c.D, c.D], BF16, kind="ExternalInput").ap()
    wk = nc.dram_tensor("wk", [c.L, c.D, c.D], BF16, kind="ExternalInput").ap()
    wv = nc.dram_tensor("wv", [c.L, c.D, c.D], BF16, kind="ExternalInput").ap()
    wo = nc.dram_tensor("wo", [c.L, c.D, c.D], BF16, kind="ExternalInput").ap()
    w1 = nc.dram_tensor("w1", [c.L, c.D, c.FF], BF16, kind="ExternalInput").ap()
    w2 = nc.dram_tensor("w2", [c.L, c.FF, c.D], BF16, kind="ExternalInput").ap()
    wout = nc.dram_tensor("wout", [c.D, c.VP], BF16, kind="ExternalInput").ap()
    logits = nc.dram_tensor("logits", [c.VP, c.SH], BF16, kind="ExternalOutput").ap()

    groups = [[2 * i, 2 * i + 1] for i in range(c.NCORES // 2)]

    with tile.TileContext(nc) as tc:
        _body(tc, c, x0T, tri, ident, rmask, peer_row,
              wq, wk, wv, wo, w1, w2, wout, logits, groups)
    nc.compile()
    return nc


def _body(tc, c, x0T, tri, ident, rmask, peer_row,
          wq, wk, wv, wo, w1, w2, wout, logits, groups):
    nc = tc.nc
    SH, FB, FFB, TB, H, HD, NSLOT = c.SH, c.FB, c.FFB, c.TB, c.H, c.HD, c.NSLOT
    AF = mybir.ActivationFunctionType
    OP = mybir.AluOpType

    ctx = ExitStack()
    pers = ctx.enter_context(tc.tile_pool(name="pers", bufs=1))
    dbl = ctx.enter_context(tc.tile_pool(name="dbl", bufs=2))
    tri_p = ctx.enter_context(tc.tile_pool(name="tri_p", bufs=4))
    olp = ctx.enter_context(tc.tile_pool(name="olp", bufs=8))
    ppool = ctx.enter_context(tc.tile_pool(name="ps", bufs=4, space="PSUM"))
    ppool2 = ctx.enter_context(tc.tile_pool(name="ps2", bufs=2, space="PSUM"))
    dram = ctx.enter_context(tc.tile_pool(name="dram", bufs=2, space="DRAM"))

    def psum(part, free):
        t = ppool.tile([P, 512], F32, tag="ps", name="ps")
        return t[:part, :free]

    def psum2(part=P, free=1024):
        t = ppool2.tile([P, 1024], F32, tag="ps2", name="ps2")
        return t[:part, :free]

    def jmm(dep_row, n=128):
        """Keep-warm junk matmul: depends on a just-computed stat row so the
        scheduler threads it through the serial DVE/ACT chain, keeping the
        PE HAM activity window from ever seeing a full idle window."""
        ps = psum(P, n)
        nc.tensor.matmul(ps[:, :n], ones_colf, dep_row[0:1, 0:n],
                         start=True, stop=True)

    # ---- constants ----
    ones_red = pers.tile([P, 1], BF16, tag="ones_red")
    nc.gpsimd.memset(ones_red[:], 1.0)
    ones_col = pers.tile([1, P], BF16, tag="ones_col")
    nc.gpsimd.memset(ones_col[:], 1.0)
    ones_colf = pers.tile([1, P], F32, tag="ones_colf")
    nc.gpsimd.memset(ones_colf[:], 1.0)
    tri_sb = pers.tile([P, P], BF16, tag="tri_sb")
    nc.sync.dma_start(tri_sb[:], tri)
    ident_sb = pers.tile([P, P], BF16, tag="ident_sb")
    nc.sync.dma_start(ident_sb[:], ident)
    rmask_sb = pers.tile([P, NSLOT], F32, tag="rmask_sb")
    nc.sync.dma_start(rmask_sb[:], rmask)
    rows = pers.tile([1, 4, SH], F32, tag="rows")
    rows16 = pers.tile([1, 2, SH], BF16, tag="rows16")

    # peer row index register (sync engine; used for dynamic DMA slices)
    peer_tmp = nc.sync.alloc_register("peer_row_reg")
    nc.sync.reg_load(peer_tmp, peer_row[0:1, 0:1])
    peer = nc.sync.snap(peer_tmp, donate=True, min_val=0, max_val=1)

    # ---- residual stream: fp32 + bf16 shadow (+ shadow of squares) ----
    xT = pers.tile([P, FB, SH], F32, tag="xT")
    nc.sync.dma_start(xT[:], x0T.rearrange("(fb p) t -> p fb t", p=P))
    xb = pers.tile([P, FB, SH], BF16, tag="xb")

    def update_shadow():
        """Refresh xb (bf16 copy of xT) and xsq (xb^2) per feature block, on
        ScalarE so the chain pipelines with residual adds + stats matmuls."""
        xsq = pers.tile([P, FB, SH], BF16, tag="hT", name="xsq")
        for dblk in range(FB):
            nc.scalar.copy(xb[:, dblk, :], xT[:, dblk, :])
            nc.scalar.activation(xsq[:, dblk, :], xb[:, dblk, :], AF.Square)
        return xsq

    xsq = update_shadow()

    def ln_pass(xsq):
        """LayerNorm (g/b folded host-side): xb -> hT bf16 (feature-major)."""
        hT = pers.tile([P, FB, SH], BF16, tag="hT", name="hT")
        for (o, w) in _halves(SH):
            st0 = psum(1, w)
            st1 = psum(1, w)
            for fb in range(FB):
                nc.tensor.matmul(st0[0:1, :w], ones_red, xb[:, fb, o:o + w],
                                 start=(fb == 0), stop=(fb == FB - 1))
                nc.tensor.matmul(st1[0:1, :w], ones_red, xsq[:, fb, o:o + w],
                                 start=(fb == 0), stop=(fb == FB - 1))
            nm = rows[0:1, 0, o:o + w]     # -mean
            r1 = rows[0:1, 1, o:o + w]     # E[x^2] -> var+eps
            t2 = rows[0:1, 2, o:o + w]     # mean^2 -> 1/(var+eps)
            t3 = rows[0:1, 3, o:o + w]     # rstd
            nc.vector.tensor_scalar_mul(nm, st0[0:1, :w], -1.0 / c.D)
            nc.vector.tensor_scalar(r1, st1[0:1, :w], 1.0 / c.D, float(c.eps),
                                    OP.mult, OP.add)
            nc.scalar.activation(t2, nm, AF.Square)
            nc.vector.tensor_sub(r1, r1, t2)
            nc.vector.reciprocal_approx_fast(t2, r1)
            nc.scalar.activation(t3, t2, AF.Sqrt)
            nc.vector.tensor_mul(nm, nm, t3)   # -mean*rstd
            nc.vector.tensor_copy(rows16[0:1, 0, o:o + w], t3)
            nc.vector.tensor_copy(rows16[0:1, 1, o:o + w], nm)
            bc = psum2()
            rb_ps = bc[:, 0:w]
            mb_ps = bc[:, 512:512 + w]
            nc.tensor.matmul(rb_ps[:, :w], ones_col, rows16[0:1, 0, o:o + w],
                             start=True, stop=True)
            nc.tensor.matmul(mb_ps[:, :w], ones_col, rows16[0:1, 1, o:o + w],
                             start=True, stop=True)
            rb_sb = dbl.tile([P, 512], BF16, tag="rb_sb", name="rb_sb")
            mb_sb = dbl.tile([P, 512], BF16, tag="mb_sb", name="mb_sb")
            nc.scalar.copy(rb_sb[:, :w], rb_ps[:, :w])
            nc.scalar.copy(mb_sb[:, :w], mb_ps[:, :w])
            for fb in range(FB):
                nc.vector.tensor_mul(hT[:, fb, o:o + w], xb[:, fb, o:o + w],
                                     rb_sb[:, :w])
                nc.vector.tensor_add(hT[:, fb, o:o + w], hT[:, fb, o:o + w],
                                     mb_sb[:, :w])
        return hT

    def proj_featmajor(hT, w_sb, store):
        """store(dblk, o, w, ps) with ps = (w.T @ h)[dblk][:, o:o+w]."""
        for dblk in range(FB):
            pss = [psum(P, w) for (o, w) in _halves(SH)]
            for ks in range(FB):
                for hi, (o, w) in enumerate(_halves(SH)):
                    nc.tensor.matmul(pss[hi][:, :w],
                                     w_sb[:, ks, dblk * P:(dblk + 1) * P],
                                     hT[:, ks, o:o + w],
                                     start=(ks == 0), stop=(ks == FB - 1))
            for hi, (o, w) in enumerate(_halves(SH)):
                store(dblk, o, w, pss[hi])

    def proj_featmajor_hfirst(hT, w_sb, store):
        """Column-half-outer variant: completes half 0 for all dblk first so
        its consumers overlap the LN row math of half 1."""
        for hi, (o, w) in enumerate(_halves(SH)):
            for dblk in range(FB):
                ps = psum(P, w)
                for ks in range(FB):
                    nc.tensor.matmul(ps[:, :w],
                                     w_sb[:, ks, dblk * P:(dblk + 1) * P],
                                     hT[:, ks, o:o + w],
                                     start=(ks == 0), stop=(ks == FB - 1))
                store(dblk, o, w, ps)

    KW = FB * SH
    VW = TB * H * (HD + 1)

    for l in range(c.L):
        # ================= attention sublayer =================
        hT = ln_pass(xsq)

        # K projection first -> feeds the AllGather asap
        wk_sb = dbl.tile([P, FB, c.D], BF16, tag="wmat", name="wk_sb")
        nc.sync.dma_start(wk_sb[:], wk[l].rearrange("(ks p) n -> p ks n", p=P))
        kT_sb = pers.tile([P, FB, SH], BF16, tag="kT_sb", name="kT_sb")
        proj_featmajor_hfirst(hT, wk_sb,
                              lambda dblk, o, w, ps: nc.scalar.copy(
                                  kT_sb[:, dblk, o:o + w], ps[:, :w]))

        # V projection (token-major, padded with ones column for denominators)
        wv_sb = dbl.tile([P, FB, c.D], BF16, tag="wmat", name="wv_sb")
        nc.sync.dma_start(wv_sb[:], wv[l].rearrange("(ks p) n -> p ks n", p=P))
        v_sb = pers.tile([P, TB, H, HD + 1], BF16, tag="v_sb", name="v_sb")
        nc.gpsimd.memset(v_sb[:, :, :, HD:HD + 1], 1.0)
        for tb in range(TB):
            ps = psum(P, c.D)
            for ks in range(FB):
                nc.tensor.matmul(ps[:, :c.D], hT[:, ks, tb * P:(tb + 1) * P],
                                 wv_sb[:, ks, :], start=(ks == 0),
                                 stop=(ks == FB - 1))
            nc.vector.tensor_copy(
                v_sb[:, tb, :, 0:HD],
                ps[:, :c.D].rearrange("p (h d) -> p h d", h=H))

        # pack + AllGather K/V across the pair (async; local pass overlaps)
        kv_d = dram.tile([P, KW + VW], BF16, tag="kv_d")
        kvg = dram.tile([2 * P, KW + VW], BF16, tag="kvg")
        nc.sync.dma_start(kv_d[:, :KW].rearrange("p (fb t) -> p fb t", fb=FB),
                          kT_sb[:])
        nc.sync.dma_start(
            kv_d[:, KW:].rearrange("p (tb h d) -> p tb h d", tb=TB, h=H),
            v_sb[:])
        if os.environ.get('NO_COLLECTIVE'):
            nc.sync.dma_start(kvg[:P], kv_d[:])
            nc.sync.dma_start(kvg[P:], kv_d[:])
        else:
            nc.gpsimd.collective_compute(
                "AllGather", OP.bypass, replica_groups=groups,
                ins=[kv_d[:].opt()], outs=[kvg[:].opt()])

        # prefetch the big FFN weight while the collective flies
        w1_sb = pers.tile([P, FB, c.FF], BF16, tag="w1", name="w1_sb")
        nc.sync.dma_start(w1_sb[:], w1[l].rearrange("(ks p) n -> p ks n", p=P))

        # Q projection
        wq_sb = dbl.tile([P, FB, c.D], BF16, tag="wmat", name="wq_sb")
        nc.sync.dma_start(wq_sb[:], wq[l].rearrange("(ks p) n -> p ks n", p=P))
        qT = pers.tile([P, FB, SH], BF16, tag="qT", name="qT")
        proj_featmajor(hT, wq_sb,
                       lambda dblk, o, w, ps: nc.scalar.copy(
                           qT[:, dblk, o:o + w], ps[:, :w]))

        # peer K/V destination (DMA waits on the collective via kvg dep);
        # shares the aT slot: aT(l) is written only after kvr(l)'s last read
        kvr = pers.tile([P, KW + VW], BF16, tag="aT", name="kvr")
        nc.sync.dma_start(kvr[:], kvg[bass.ts(peer, P), :])
        kTr = kvr[:, 0:KW]
        vr = kvr[:, KW:KW + VW]

        # ---- attention: LOCAL pass then REMOTE pass, 2 heads interleaved,
        # one PSUM bank per (head, column-half) unit; score/exp units lead
        # the AV matmuls by SKEW units so AV never waits on exp+mask ----
        oT = pers.tile([P, FB, SH], BF16, tag="oT", name="oT")
        oL = {}
        SKEW = 2

        def attn_pair(phase, hp):
            o_ps = {}
            for hh in (0, 1):
                h = 2 * hp + hh
                for hi, (o, w) in enumerate(_halves(SH)):
                    o_ps[(hh, hi)] = psum(HD + 1, w)
                    if phase == 1:
                        nc.tensor.matmul(o_ps[(hh, hi)][:, :w],
                                         ident_sb[0:HD + 1, 0:HD + 1],
                                         oL[h][:, o:o + w],
                                         start=True, stop=False)
            units = []
            for ks in reversed(range(NSLOT)):
                for hh in (0, 1):
                    units.append((ks, hh))
            pTq = {}

            def emit_front(idx):
                ks, hh = units[idx]
                mq = (ks + 1) * P
                rh = hh * HD
                # full-chunk score tile (2 banks): 2 matmuls but ONE exp
                sc = psum2(P, mq)
                if phase == 0:
                    lhs = kT_sb[rh:rh + HD, hp, ks * P:(ks + 1) * P]
                else:
                    lhs = kTr[rh:rh + HD,
                              hp * SH + ks * P:hp * SH + (ks + 1) * P]
                for (o, w) in _halves(mq):
                    nc.tensor.matmul(sc[:, o:o + w], lhs,
                                     qT[rh:rh + HD, hp, o:o + w],
                                     start=True, stop=True)
                pT = tri_p.tile([P, 1024], BF16, tag="pT", name="pT")
                nc.scalar.activation(pT[:, :mq], sc[:, :mq], AF.Exp,
                                     scale=float(c.inv_scale))
                if phase == 0:
                    nc.vector.tensor_mul(pT[:, ks * P:(ks + 1) * P],
                                         pT[:, ks * P:(ks + 1) * P], tri_sb[:])
                else:
                    nc.vector.tensor_scalar_mul(pT[:, ks * P:(ks + 1) * P],
                                                pT[:, ks * P:(ks + 1) * P],
                                                rmask_sb[:, ks:ks + 1])
                pTq[idx] = pT

            def emit_av(idx):
                ks, hh = units[idx]
                mq = (ks + 1) * P
                h = 2 * hp + hh
                if phase == 0:
                    vlhs = v_sb[:, ks, h, :]
                else:
                    voff = ks * H * (HD + 1) + h * (HD + 1)
                    vlhs = vr[:, voff:voff + HD + 1]
                pT = pTq.pop(idx)
                for hi, (o, w) in enumerate(_halves(mq)):
                    nc.tensor.matmul(o_ps[(hh, hi)][:, :w], vlhs,
                                     pT[:, o:o + w],
                                     start=(phase == 0 and ks == NSLOT - 1),
                                     stop=(ks == o // P))

            for idx in range(len(units) + SKEW):
                if idx < len(units):
                    emit_front(idx)
                if idx >= SKEW:
                    emit_av(idx - SKEW)
            return o_ps

        def attn_tail(phase, hp, o_ps):
            if phase == 0:
                for hh in (0, 1):
                    h = 2 * hp + hh
                    oL[h] = olp.tile([HD + 1, SH], BF16, tag="oL",
                                     name=f"oL{h}")
                    for hi, (o, w) in enumerate(_halves(SH)):
                        nc.vector.tensor_copy(oL[h][:, o:o + w],
                                              o_ps[(hh, hi)][:HD + 1, :w])
            else:
                # normalize: recip of denominators, broadcast, scale
                for hh in (0, 1):
                    rh = hh * HD
                    sums = rows[0:1, 2 + hh, :SH]
                    recip = rows[0:1, hh, :SH]
                    for hi, (o, w) in enumerate(_halves(SH)):
                        nc.vector.tensor_copy(sums[0:1, o:o + w],
                                              o_ps[(hh, hi)][HD:HD + 1, :w])
                    nc.vector.reciprocal_approx_fast(recip, sums)
                    nc.vector.tensor_copy(rows16[0:1, hh, :SH], recip)
                    rbt = psum2(HD, 1024)
                    for hi, (o, w) in enumerate(_halves(SH)):
                        rb = rbt[:HD, o:o + w]
                        nc.tensor.matmul(rb[:HD, :w], ones_col[0:1, 0:HD],
                                         rows16[0:1, hh, o:o + w],
                                         start=True, stop=True)
                        rb_sb = dbl.tile([HD, 512], F32, tag="rb2",
                                         name="rb_sb2")
                        nc.vector.tensor_copy(rb_sb[:, :w], rb[:HD, :w])
                        nc.vector.tensor_mul(oT[rh:rh + HD, hp, o:o + w],
                                             o_ps[(hh, hi)][:HD, :w],
                                             rb_sb[:, :w])

        for phase in (0, 1):            # 0 = local, 1 = remote
            for hp in range(H // 2):
                o_ps = attn_pair(phase, hp)
                attn_tail(phase, hp, o_ps)

        # ---- wo projection + residual ----
        wo_sb = dbl.tile([P, FB, c.D], BF16, tag="wmat", name="wo_sb")
        nc.sync.dma_start(wo_sb[:], wo[l].rearrange("(ks p) n -> p ks n", p=P))

        def store_attn(dblk, o, w, ps):
            nc.vector.tensor_add(xT[:, dblk, o:o + w], xT[:, dblk, o:o + w],
                                 ps[:, :w])

        proj_featmajor(oT, wo_sb, store_attn)
        xsq = update_shadow()

        # ================= FFN sublayer =================
        h2T = ln_pass(xsq)
        aT = pers.tile([P, FFB, SH], BF16, tag="aT", name="aT")
        for hi, (o, w) in enumerate(_halves(SH)):
            for fb in range(FFB):
                ps = psum(P, w)
                for ks in range(FB):
                    nc.tensor.matmul(ps[:, :w],
                                     w1_sb[:, ks, fb * P:(fb + 1) * P],
                                     h2T[:, ks, o:o + w],
                                     start=(ks == 0), stop=(ks == FB - 1))
                if (fb + hi) % 2 == 0:
                    nc.scalar.activation(aT[:, fb, o:o + w], ps[:, :w],
                                         AF.Relu)
                else:
                    nc.vector.tensor_scalar_max(aT[:, fb, o:o + w],
                                                ps[:, :w], 0.0)

        W2C = 4
        # 8 accumulator banks: dblk 0/1 in the two wide slots, 2/3 in halves
        y_ps = {}
        for dblk in range(2):
            yw = psum2(P, 1024)
            for hi, (o, w) in enumerate(_halves(SH)):
                y_ps[dblk * 2 + hi] = yw[:, o:o + w]
        for dblk in range(2, FB):
            for hi, (o, w) in enumerate(_halves(SH)):
                y_ps[dblk * 2 + hi] = psum(P, w)
        for fc in range(0, FFB, W2C):
            w2_sb = dbl.tile([P, W2C, c.D], BF16, tag="w2c", name="w2_sb")
            nc.sync.dma_start(
                w2_sb[:],
                w2[l, fc * P:(fc + W2C) * P].rearrange("(fb p) n -> p fb n", p=P))
            for fb_ in range(W2C):
                fb = fc + fb_
                for dblk in range(FB):
                    for hi, (o, w) in enumerate(_halves(SH)):
                        nc.tensor.matmul(
                            y_ps[dblk * 2 + hi][:, :w],
                            w2_sb[:, fb_, dblk * P:(dblk + 1) * P],
                            aT[:, fb, o:o + w],
                            start=(fb == 0), stop=(fb == FFB - 1))
        for dblk in range(FB):
            for hi, (o, w) in enumerate(_halves(SH)):
                nc.vector.tensor_add(xT[:, dblk, o:o + w],
                                     xT[:, dblk, o:o + w],
                                     y_ps[dblk * 2 + hi][:, :w])
        xsq = update_shadow()

    # ================= final LN + vocab projection =================
    hT = ln_pass(xsq)
    VCW = (c.NVB // c.VCH) * P          # vocab cols per DMA chunk
    for vc in range(c.VCH):
        wch = dbl.tile([P, FB, VCW], BF16, tag="wmat", name="wch")
        nc.sync.dma_start(
            wch[:],
            wout[:, vc * VCW:(vc + 1) * VCW].rearrange("(ks p) n -> p ks n", p=P))
        for vb in range(VCW // P):
            pss = [psum(P, w) for (o, w) in _halves(SH)]
            for ks in range(FB):
                for hi, (o, w) in enumerate(_halves(SH)):
                    nc.tensor.matmul(pss[hi][:, :w],
                                     wch[:, ks, vb * P:(vb + 1) * P],
                                     hT[:, ks, o:o + w],
                                     start=(ks == 0), stop=(ks == FB - 1))
            lg = dbl.tile([P, SH], BF16, tag="lg", name="lg")
            for hi, (o, w) in enumerate(_halves(SH)):
                if (vb + hi) % 2 == 0:
                    nc.vector.tensor_copy(lg[:, o:o + w], pss[hi][:, :w])
                else:
                    nc.scalar.copy(lg[:, o:o + w], pss[hi][:, :w])
            row0 = (vc * (VCW // P) + vb) * P
            nc.sync.dma_start(logits[row0:row0 + P, :], lg[:])

    ctx.close()


# ---------------- host side ----------------

def _pos_encoding(S, D):
    pos = np.arange(S, dtype=np.float32)[:, None]
    div = np.exp(np.arange(0, D, 2, dtype=np.float32) * (-np.log(10000.0) / D))
    pe = np.zeros((S, D), np.float32)
    pe[:, 0::2] = np.sin(pos * div)
    pe[:, 1::2] = np.cos(pos * div)
    return pe


def _bf(x):
    return np.asarray(x, np.float32).astype(ml_dtypes.bfloat16)


def make_inputs(c, tokens, embed, wq, wk, wv, wo, bo, w1, b1, w2, b2,
                ln_g, ln_b, lnf_g, lnf_b, w_out, b_out):
    """Host preprocessing -> per-core input maps + host-side output row."""
    tokens = np.asarray(tokens)
    x0 = np.asarray(embed, np.float32)[tokens] + _pos_encoding(c.S, c.D)[None]
    ln_g = np.asarray(ln_g, np.float32)
    assert not np.any(np.asarray(ln_b)), "nonzero ln_b unsupported"
    assert not np.any(np.asarray(bo)), "nonzero bo unsupported"
    assert not np.any(np.asarray(b1)), "nonzero b1 unsupported"
    assert not np.any(np.asarray(b2)), "nonzero b2 unsupported"
    wq_f = _bf(np.asarray(wq, np.float32) * ln_g[:, :, None])
    wk_f = _bf(np.asarray(wk, np.float32) * ln_g[:, :, None])
    wv_f = _bf(np.asarray(wv, np.float32) * ln_g[:, :, None])
    w1_f = _bf(np.asarray(w1, np.float32) * ln_g[:, :, None])
    wo_f = _bf(wo)
    w2_f = _bf(w2)
    wout_full = np.asarray(w_out, np.float32) * np.asarray(lnf_g, np.float32)[:, None]
    wout_pad = np.zeros((c.D, c.VP), np.float32)
    wout_pad[:, :c.V] = wout_full
    wout_f = _bf(wout_pad)
    out_row = (np.asarray(lnf_b, np.float32) @ np.asarray(w_out, np.float32)
               + np.asarray(b_out, np.float32))
    assert not np.any(lnf_b), "nonzero lnf_b unsupported"

    tri_m = (np.arange(P)[:, None] <= np.arange(P)[None, :]).astype(
        ml_dtypes.bfloat16)
    ident_m = np.eye(P, dtype=np.float32).astype(ml_dtypes.bfloat16)

    in_maps = []
    for core in range(c.NCORES):
        r, bp = core % 2, core // 2
        chunks = c.slot_chunks[r]
        tok_rows = np.concatenate(
            [np.arange(ch * P, (ch + 1) * P) for ch in chunks])
        x0T = np.ascontiguousarray(x0[bp, tok_rows, :].T.astype(np.float32))
        in_maps.append({
            "x0T": x0T,
            "tri": tri_m,
            "ident": ident_m,
            "rmask": np.repeat(c.rmask[r][None, :], P, axis=0).astype(np.float32),
            "peer_row": np.array([[1 - r]], np.uint32),
            "wq": wq_f, "wk": wk_f, "wv": wv_f, "wo": wo_f,
            "w1": w1_f, "w2": w2_f,
            "wout": wout_f,
        })
    return in_maps, out_row


def gather_output(c, results, out_row):
    out = np.zeros((c.B, c.S, c.V), np.float32)
    for core in range(c.NCORES):
        r, bp = core % 2, core // 2
        chunks = c.slot_chunks[r]
        lg = np.asarray(results[core]["logits"]).astype(np.float32)  # [VP, SH]
        for s, ch in enumerate(chunks):
            out[bp, ch * P:(ch + 1) * P, :] = lg[:c.V, s * P:(s + 1) * P].T
    if np.any(out_row):
        out += out_row[None, None, :]
    return out


_CACHE = {}


def run(inputs, trace=False):
    c = make_cfg()
    in_maps, out_row = make_inputs(c, **inputs)
    if "nc" not in _CACHE:
        _CACHE["nc"] = build_program(c)
    res = run_bass_kernel_spmd(_CACHE["nc"], in_maps,
                               core_ids=list(range(c.NCORES)), trace=trace)
    return gather_output(c, res.results, out_row), res


def kernel(**inputs):
    return run(inputs)[0]
